# revision 75
# baseline (speedup 1.0000x reference)
"""Trainium2 Bass kernel for nn_ActorGraphPolicy (tree message-passing policy).

Pure data-parallel: batch 32768 sharded across 8 NeuronCores (4096 rows each).
Per-core program processes the batch in chunks of 512 columns, with all
activations kept feature-major ([feature, batch_cols]) in SBUF so every matmul
contracts over the partition dimension.

TRN2 engine ops require 32-aligned partition bases, so concatenated inputs use
padded layouts whose pad rows carry zero weights:
  cs tile [107, CB]: |dpos|@0, dpos@32, parent_pos@64, child_state@96 (11 rows)
  l1 input: xa1 = tanh(cs[0:107]) (rel part), xa2 = [tanh(mu); tanh(msg_in)]

Pipeline structure: the top-down pass of chunk c is interleaved at tree-level
granularity with the bottom-up pass of chunk c+1 (and chunk c+2's front at
the tail), every interleave step ending in ONE shared Abs_reciprocal_sqrt
phase whose sum-of-squares land at 32-aligned slots of shared PSUM tiles —
one ACT table round-trip per step, and each track's serial norm latency is
covered by the other track's independent matmuls (keeps the PE HAM-warm).
Down levels run a skewed wavefront (stage j of node i at key i+2(j-1)) so
sibling nodes' matmuls cover each other's PSUM-drain latency.  The up-pass
fc1/fc2/fc3 are pair-packed block-diagonal over node pairs (2 nodes per
matmul); attention logit pairs share one PSUM bank in concurrent col-halves.
"""
import os
import numpy as np

import concourse.bass as bass
import concourse.tile as tile
from concourse import bacc, mybir
from concourse.bass_utils import run_bass_kernel_spmd

AF = mybir.ActivationFunctionType
OP = mybir.AluOpType
F32 = mybir.dt.float32
F32R = mybir.dt.float32r
FP8 = mybir.dt.float8e4
DRPM = mybir.MatmulPerfMode.DoubleRow

PARENTS = [-1, 0, 0, 1, 1, 2, 2, 3, 4, 5, 6, 7]
NL, SD, MD = 12, 11, 64
CHILDREN = [[i for i, p in enumerate(PARENTS) if p == n] for n in range(NL)]
SLOT = [PARENTS[:n].count(PARENTS[n]) for n in range(NL)]  # child slot index
BATCH = 32768
NCORES = 8
BLOC = BATCH // NCORES  # 4096
CB = 512                # batch columns per chunk
EPS = 1e-12

MM_DT = os.environ.get("MM_DT", "bf16")  # 'f32' | 'bf16'
NCH = int(os.environ.get("NCH", BLOC // CB))
SN_BUFS = int(os.environ.get("SN_BUFS", 2))
BF16 = mybir.dt.bfloat16
MDT = F32 if MM_DT == "f32" else BF16  # dtype of matmul-feeding tiles/weights

UP_ORDER = list(range(NL - 1, -1, -1))
# l1 output layout (896 packed cols, 7 x 128 for DoubleRow col_grp=0xf):
# j0-2 act[0:384]; j3 = [act tail 16 | one | 0*15 | msg tail 16 | one | 0*79];
# j4-6 msg[0:384].
M_J = [128] * 7
KS4 = [128, 128, 128, 32]
H2_KS = [128, 128, 45]
LEAVES = {8, 9, 10, 11}       # msg_down of leaves is never consumed
DOWN_LEVELS = [[0], [1, 2], [3, 4, 5, 6], [7, 8, 9, 10], [11]]


def _mm_in(ap):
    return ap


def build_program(nch=NCH):
    nc = bacc.Bacc("TRN2", target_bir_lowering=False)

    def din(name, shape):
        return nc.dram_tensor(name, shape, F32, kind="ExternalInput")

    def dinm(name, shape):
        return nc.dram_tensor(name, shape, MDT, kind="ExternalInput")

    statet = dinm("statet", [132, nch * CB])
    w1 = dinm("w1", [12, 64])  # row 11 = fc1 bias (SN row 11 == 1)
    w1p = dinm("w1p", [23, 128])  # pair block-diag fc1, row 22 = bias
    wattp = dinm("wattp", [107, 64]); wattn = dinm("wattn", [107, 64])
    def dinq(name, shape):
        return nc.dram_tensor(name, shape, FP8, kind="ExternalInput")

    w2dr = dinq("w2dr", [64, 2, 128]); b2e = din("b2e", [65, 1])  # col64: h2 one-row
    w3 = dinm("w3", [65, 64])  # row 64 = fc3 bias (h2 row 64 == 1)
    w2p = dinq("w2p", [128, 2, 128]); b2p = din("b2p", [128, 1])
    w3p = dinm("w3p", [128, 128]); b3p = din("b3p", [128, 1])
    ones64 = dinm("ones64", [64, 1]);   onesb64 = dinm("onesb64", [1, 64])
    ones128 = dinm("ones128", [128, 1])
    sel2 = dinm("sel2", [128, 2])
    selb2x = dinm("selb2x", [98, 128])  # SELB2 pattern at bases 0/32/64/96
    onesbx = dinm("onesbx", [98, 128])  # all-ones; sliced at any 32-base
    # act head l1 stays bf16 (448 packed cols: act[0:384] | j3 tails 64)
    wl1a_b = dinm("wl1a_b", [107, 448])
    wl1b_b = dinm("wl1b_b", [128, 448])
    # msg head l1 in fp8 DoubleRow (j4-j6, 3 x 128 cols)
    wl1drm = dinq("wl1drm", [128, 2, 384])
    # act head l2 bf16 (4 k-chunks), msg head l2 fp8 DoubleRow
    wl2a = [dinm(f"wl2a{i}", [KS4[i], 301]) for i in range(4)]
    wl2drm = [dinq(f"wl2drm{i}", [128, 2, 384]) for i in range(2)]
    wl3a = [dinm(f"wl3a{i}", [H2_KS[i], 1]) for i in range(3)]
    wl3m = [dinm(f"wl3m{i}", [H2_KS[i], 128]) for i in range(3)]
    outt = nc.dram_tensor("outt", [12, nch * CB], F32, kind="ExternalOutput")

    with tile.TileContext(nc) as tc:
        with (
            nc.allow_low_precision(reason="bf16 matmul inputs; PSUM accumulates fp32"),
            tc.tile_pool(name="wp", bufs=1) as wp,          # weights, persistent
            tc.tile_pool(name="pp", bufs=1) as pp,          # per-chunk persistent
            tc.tile_pool(name="tp", bufs=1) as tp,          # transients
            tc.tile_pool(name="pbig", bufs=5, space="PSUM") as pbig,
            tc.tile_pool(name="pnrm", bufs=1, space="PSUM") as pnrm,
            tc.tile_pool(name="psm", bufs=2, space="PSUM") as psm,
        ):
            def wload(dram, shape, tag, dt=MDT):
                t = wp.tile(shape, dt, tag=tag, name=tag)
                nc.sync.dma_start(t[:], dram[:])
                return t

            W1P = wload(w1p, [23, 128], "W1P")
            WATTP = wload(wattp, [107, 64], "WATTP")
            WATTN = wload(wattn, [107, 64], "WATTN")
            W2DR = wload(w2dr, [64, 2, 128], "W2DR", dt=FP8)
            B2E = wload(b2e, [65, 1], "B2E", dt=F32)
            W3 = wload(w3, [65, 64], "W3")
            ON64 = wload(ones64, [64, 1], "ON64");  OB64 = wload(onesb64, [1, 64], "OB64")
            ON128 = wload(ones128, [128, 1], "ON128")
            SEL2 = wload(sel2, [128, 2], "SEL2")
            ONER = wp.tile([1, CB], MDT, tag="ONER", name="ONER")
            nc.gpsimd.memset(ONER[:], 1.0)
            TENR = wp.tile([1, CB], MDT, tag="TENR", name="TENR")
            nc.gpsimd.memset(TENR[:], 10.0)
            SCR = wp.tile([1, 64], F32, tag="SCR", name="SCR")
            nc.gpsimd.memset(SCR[:], 0.0)
            W2P = wload(w2p, [128, 2, 128], "W2P", dt=FP8)
            B2P = wload(b2p, [128, 1], "B2P", dt=F32)
            W3P = wload(w3p, [128, 128], "W3P")
            B3P = wload(b3p, [128, 1], "B3P", dt=F32)
            SELB2X = wload(selb2x, [98, 128], "SELB2X")
            ONESBX = wload(onesbx, [98, 128], "ONESBX")
            WL1AB = wload(wl1a_b, [107, 448], "WL1AB")
            WL1BB = wload(wl1b_b, [128, 448], "WL1BB")
            WL1DRM = wload(wl1drm, [128, 2, 384], "WL1DRM", dt=FP8)
            WL2A = [wload(wl2a[i], [KS4[i], 301], f"WL2A{i}") for i in range(4)]
            WL2DRM = [wload(wl2drm[i], [128, 2, 384], f"WL2DRM{i}", dt=FP8)
                      for i in range(2)]
            WL3A = [wload(wl3a[i], [H2_KS[i], 1], f"WL3A{i}") for i in range(3)]
            WL3M = [wload(wl3m[i], [H2_KS[i], 128], f"WL3M{i}") for i in range(3)]

            _zinit = {}

            def zonce(tag, aps, nbufs=2):
                """Zero pad-regions of a tile tag on its first nbufs
                allocations; the regions are never written afterwards."""
                k = _zinit.get(tag, 0)
                if k < nbufs:
                    for ap in aps:
                        nc.gpsimd.memset(ap, 0.0)
                    _zinit[tag] = k + 1

            UP_PAIRS = [(11, 10), (9, 8), (7, 6), (5, 4), (3, 2), (1, 0)]
            # pair groups whose mu-norms batch into one rsqrt phase
            UP_GROUPS = [[0, 1], [2, 3], [4]]  # indices into UP_PAIRS; (1,0) special

            def build_cs(SN, dst, nfrom, nto, with_rest):
                """dst[0:3]=|d|, [32:35]=d=pos(nfrom)-pos(nto), [64:67]=pos(nto),
                [96:96+r]=state(nfrom)."""
                nc.vector.tensor_sub(dst[32:35], SN[nfrom][0:3], SN[nto][0:3])
                nc.scalar.activation(dst[0:3], dst[32:35], AF.Abs)
                nc.vector.tensor_copy(dst[64:67], SN[nto][0:3])
                nc.vector.tensor_copy(dst[96:96 + (SD if with_rest else 3)],
                                      SN[nfrom][0:SD if with_rest else 3])

            # ---- shared norm-phase machinery ------------------------------
            # All l2-norm sum-of-squares land in shared [98, CB] PSUM tiles
            # at 32-aligned slot bases; ONE Abs_reciprocal_sqrt ACTIVATE per
            # tile covers every slot (garbage between slots is computed but
            # never read).  This makes each interleave step cost exactly one
            # sqrt-table round-trip and keeps the phase atomic against the
            # scheduler slotting tanh/sigmoid work mid-phase.
            def ph_new():
                return dict(tiles=[], minvts=[], used=0)

            def ph_slot(ph, nrows):
                k, ph["used"] = ph["used"], ph["used"] + 1
                ti, sl = divmod(k, 3)  # matmul out base partition must be <=64
                if ti == len(ph["tiles"]):
                    ph["tiles"].append(psm.tile([98, CB], F32, tag="S",
                                                name=f"ph{ti}"))
                    ph["minvts"].append(
                        tp.tile([98, CB], MDT, tag=f"minv{ti}",
                                name=f"minv{ti}", bufs=1))
                ph.setdefault("hi", {})[ti] = 32 * sl + nrows
                return ph["tiles"][ti], 32 * sl, ph["minvts"][ti]

            def ph_rsqrt(ph):
                for ti, pt in enumerate(ph["tiles"]):
                    rows = ph["hi"][ti]
                    nc.scalar.activation(ph["minvts"][ti][0:rows], pt[0:rows],
                                         AF.Abs_reciprocal_sqrt)
                if ph["tiles"]:
                    # dummy tanh: pulls the sqrt->tanh ACT table switch off
                    # the critical path (it runs during the bcast/mul
                    # latency instead of gating the first real xa2 tanh)
                    nc.scalar.activation(SCR[0:1, 0:16], SCR[0:1, 0:16],
                                         AF.Tanh)

            def emit_front_mm(c, ph):
                """Input DMAs + hoisted fc1 + x-norm sumsq for chunk c (the
                rsqrts batch into the step's shared ACT phase)."""
                ccols = slice(c * CB, (c + 1) * CB)
                SN = [pp.tile([SD + 1, CB], MDT, tag=f"sn{n}", name=f"sn{n}",
                              bufs=SN_BUFS) for n in range(NL)]
                for n in range(NL):
                    nc.sync.dma_start(SN[n][0:SD, :], statet[SD * n:SD * (n + 1), ccols])
                # ---- hoisted pair-packed fc1 + x-norm sumsq (6 pairs) ----
                XHP = []
                slots = []
                for pi, (na, nb) in enumerate(UP_PAIRS):
                    k = nb // 2
                    snp = pp.tile([23, CB], MDT, tag=f"snp{k}", name=f"snp{k}")
                    nc.sync.dma_start(snp[0:22, :], statet[22 * k:22 * k + 22,
                                                           ccols])
                    if c < 1:
                        nc.sync.dma_start(snp[22:23, :], ONER[:])
                    xhp = tp.tile([128, CB], MDT, tag=f"xhp{pi}", name=f"xhp{pi}")
                    px = pbig.tile([128, CB], F32, tag="P", name="px")
                    nc.tensor.matmul(px[:], _mm_in(W1P[:]), _mm_in(snp[:]),
                                     start=True, stop=True)
                    nc.vector.tensor_copy(xhp[:], px[:])
                    XHP.append(xhp)
                for pi in range(6):
                    sqp = tp.tile([128, CB], MDT, tag="sqp", name="sqp", bufs=2)
                    nc.gpsimd.tensor_mul(sqp[:], XHP[pi][:], XHP[pi][:])
                    pt, r0, mt = ph_slot(ph, 2)
                    slots.append((r0, mt))
                    nc.tensor.matmul(pt[r0:r0 + 2], _mm_in(SEL2[:]), _mm_in(sqp[:]),
                                     start=True, stop=True)
                return dict(c=c, ccols=ccols, SN=SN, XHP=XHP, slots=slots)

            def emit_front_post(ctx):
                XNP = []
                for pi in range(6):
                    r0, mt = ctx["slots"][pi]
                    pb2 = pnrm.tile([128, CB], F32, tag="N", name="pb2")
                    nc.tensor.matmul(pb2[:], _mm_in(SELB2X[r0:r0 + 2]),
                                     _mm_in(mt[r0:r0 + 2]), start=True, stop=True)
                    nc.vector.tensor_mul(ctx["XHP"][pi][:], ctx["XHP"][pi][:],
                                         pb2[:])
                    XNP.append(ctx["XHP"][pi])
                ctx["XNP"] = XNP

            def emit_front_b(ctx):
                """cs tiles + attention gates for ctx's chunk (split out so it
                can fill TensorE gaps at a different pipeline point)."""
                c, SN = ctx["c"], ctx["SN"]
                CS, AT = {}, {}
                for n_ in range(NL):
                    for c_i in CHILDREN[n_]:
                        cst = pp.tile([107, CB], MDT, tag=f"cs{c_i}", name=f"cs{c_i}",
                                      bufs=2)
                        CS[c_i] = cst
                        if c < 2:
                            nc.gpsimd.memset(cst[:], 0.0)
                            nc.sync.dma_start(cst[35:36, :], TENR[:])
                        build_cs(SN, cst, c_i, n_, True)
                # attention logits pair-packed into [128] PSUM halves
                # (col-tiles run concurrently); one sigmoid per parent pair
                att_jobs = []
                for pa_, pb_ in ((0, 1), (2, 3), (4, 5), (6, 7)):
                    p_ = pbig.tile([128, CB], F32, tag="P", name="plpre")
                    for half, n_ in enumerate((pa_, pb_)):
                        ch_ = CHILDREN[n_]
                        b = 64 * half
                        if len(ch_) == 2:
                            nc.tensor.matmul(p_[b:b + 64], _mm_in(WATTP[:]),
                                             _mm_in(CS[ch_[0]][:]),
                                             start=True, stop=False)
                            nc.tensor.matmul(p_[b:b + 64], _mm_in(WATTN[:]),
                                             _mm_in(CS[ch_[1]][:]),
                                             start=False, stop=True)
                        else:
                            nc.tensor.matmul(p_[b:b + 64], _mm_in(WATTP[:]),
                                             _mm_in(CS[ch_[0]][:]),
                                             start=True, stop=True)
                    for half, n_ in enumerate((pa_, pb_)):
                        at = tp.tile([64, CB], MDT, tag=f"at{n_}",
                                     name=f"at{n_}", bufs=2)
                        att_jobs.append((at, p_[64 * half:64 * half + 64]))
                        AT[n_] = at
                for at, psrc in att_jobs:  # contiguous sigmoid phase
                    nc.scalar.activation(at[:], psrc, AF.Sigmoid)
                rootcs = pp.tile([107, CB], MDT, tag="rootcs", name="rootcs", bufs=2)
                if c < 2:
                    nc.gpsimd.memset(rootcs[:], 0.0)
                    nc.sync.dma_start(rootcs[35:36, :], TENR[:])
                build_cs(SN, rootcs, 0, NL - 1, False)
                ctx["CS"] = CS
                ctx["AT"] = AT
                ctx["rootcs"] = rootcs

            def up_node_pre(ctx, n, tx):
                """Blend + xm tanh for one node; returns the fp8 xm tile."""
                MU, AT = ctx["MU"], ctx["AT"]
                ch = CHILDREN[n]
                r = 64 * (n % 2)
                tag = "xm" if ch else "xml"
                xm = tp.tile([64, 2, CB], FP8, tag=tag, name=tag, bufs=1)
                if not ch:
                    zonce(tag, [xm[0:64, 1, :]])
                nc.scalar.activation(xm[0:64, 0, :], tx[r:r + 64], AF.Tanh)
                if ch:
                    m = tp.tile([64, CB], F32, tag="mp", name="m", bufs=2)
                    if len(ch) == 2:
                        # m = mu1 + sigmoid(l0 - l1) * (mu0 - mu1)
                        dmu = tp.tile([64, CB], F32, tag="dmu", name="dmu")
                        nc.gpsimd.tensor_sub(dmu[:], MU[ch[0]][:], MU[ch[1]][:])
                        nc.gpsimd.tensor_mul(dmu[:], AT[n][:], dmu[:])
                        nc.gpsimd.tensor_add(m[:], dmu[:], MU[ch[1]][:])
                    else:
                        nc.gpsimd.tensor_mul(m[:], AT[n], MU[ch[0]][:])
                    nc.scalar.activation(xm[0:64, 1, :], m[:], AF.Tanh)
                return xm

            def up_pair_pre(ctx, pi):
                """Pair-packed blend + xm tanhs: one fp8 [128, 2, CB] tile
                covers both nodes (node-even rows 0:64, node-odd 64:128)."""
                MU, AT = ctx["MU"], ctx["AT"]
                na, nb = UP_PAIRS[pi]
                leaf = not CHILDREN[na]  # pairs are uniformly leaf/branch
                tag = "xmpl" if leaf else "xmp"
                xmp = tp.tile([128, 2, CB], FP8, tag=tag, name=tag, bufs=2)
                if leaf:
                    zonce(tag, [xmp[:, 1, :]])
                nc.scalar.activation(xmp[0:128, 0, :], ctx["XNP"][pi][:],
                                     AF.Tanh)
                if not leaf:
                    mp = tp.tile([128, CB], F32, tag="mp", name="mp", bufs=2)
                    for n in (na, nb):
                        r = 64 * (n % 2)
                        ch = CHILDREN[n]
                        if len(ch) == 2:
                            # m = mu1 + sigmoid(l0 - l1) * (mu0 - mu1)
                            dmu = tp.tile([64, CB], F32, tag="dmu", name="dmu")
                            nc.gpsimd.tensor_sub(dmu[:], MU[ch[0]][:],
                                                 MU[ch[1]][:])
                            nc.gpsimd.tensor_mul(dmu[:], AT[n][:], dmu[:])
                            nc.gpsimd.tensor_add(mp[r:r + 64], dmu[:],
                                                 MU[ch[1]][:])
                        else:
                            nc.gpsimd.tensor_mul(mp[r:r + 64], AT[n][:],
                                                 MU[ch[0]][:])
                    nc.scalar.activation(xmp[0:128, 1, :], mp[:], AF.Tanh)
                return xmp

            def up_group_mm(ctx, pair_indices, ph):
                """Pair-packed W2/W3 (block-diag over the two nodes),
                stage-interleaved across the group's pairs; the mu-norm
                rsqrts land in the step's shared ACT phase."""
                xmps = [up_pair_pre(ctx, pi) for pi in pair_indices]
                p2s, h2s, p3s, mrps, slots = [], [], [], [], []
                for xmp in xmps:
                    p2 = pbig.tile([128, CB], F32, tag="P", name="p2")
                    nc.tensor.matmul(p2[:], _mm_in(W2P[:, :, :]),
                                     _mm_in(xmp[:, :, :]),
                                     start=True, stop=True, perf_mode=DRPM)
                    p2s.append(p2)
                for p2 in p2s:
                    h2p = tp.tile([128, CB], MDT, tag="h2p", name="h2p",
                                  bufs=2)
                    nc.scalar.activation(h2p[:], p2[:], AF.Tanh, bias=B2P[:])
                    h2s.append(h2p)
                for h2p in h2s:
                    p3 = pbig.tile([128, CB], F32, tag="P", name="p3p")
                    nc.tensor.matmul(p3[:], _mm_in(W3P[:]), _mm_in(h2p[:]),
                                     start=True, stop=True)
                    p3s.append(p3)
                for k, pi in enumerate(pair_indices):
                    mrp = tp.tile([128, CB], F32, tag=f"mrp{pi % 2}",
                                  name=f"mrp{pi % 2}")
                    nc.vector.tensor_scalar_add(mrp[:], p3s[k][:], B3P[:])
                    mrps.append(mrp)
                for k, pi in enumerate(pair_indices):
                    sqm = tp.tile([128, CB], MDT, tag="sqm", name="sqm", bufs=2)
                    nc.gpsimd.tensor_mul(sqm[:], mrps[k][:], mrps[k][:])
                    pt, r0, mt = ph_slot(ph, 2)
                    slots.append((r0, mt))
                    nc.tensor.matmul(pt[r0:r0 + 2], _mm_in(SEL2[:]),
                                     _mm_in(sqm[:]), start=True, stop=True)
                return dict(ctx=ctx, pair_indices=pair_indices, mrps=mrps,
                            slots=slots)

            def up_group_post(st):
                MU = st["ctx"]["MU"]
                for k, pi in enumerate(st["pair_indices"]):
                    na, nb = UP_PAIRS[pi]
                    r0, mt = st["slots"][k]
                    pbm = pnrm.tile([128, CB], F32, tag="N", name="pbm")
                    nc.tensor.matmul(pbm[:], _mm_in(SELB2X[r0:r0 + 2]),
                                     _mm_in(mt[r0:r0 + 2]),
                                     start=True, stop=True)
                    for n in (na, nb):
                        r = 64 * (n % 2)
                        nc.vector.tensor_mul(MU[n][:], st["mrps"][k][r:r + 64],
                                             pbm[r:r + 64])

            def up_root_main(ctx, n, ph):
                # pair (1,0): node 0's blend needs normalized mu_1, so the two
                # norms stay serial (one step each)
                xm = up_node_pre(ctx, n, ctx["XNP"][5])
                p2 = pbig.tile([128, CB], F32, tag="P", name="p2")
                nc.tensor.matmul(p2[:], _mm_in(W2DR[:, :, :]), _mm_in(xm[:, :, :]),
                                 start=True, stop=True, perf_mode=DRPM)
                h2 = tp.tile([65, CB], MDT, tag="h2u", name="h2u", bufs=1)
                nc.scalar.activation(h2[:], p2[0:65], AF.Tanh, bias=B2E[:])
                p3 = pbig.tile([64, CB], F32, tag="P", name="p3")
                nc.tensor.matmul(p3[:], _mm_in(W3[:]), _mm_in(h2[:]),
                                 start=True, stop=True)
                mr = tp.tile([64, CB], F32, tag="mr", name="mr")
                nc.vector.tensor_copy(mr[:], p3[:])
                sqr = tp.tile([64, CB], MDT, tag="sqr", name="sqr")
                nc.gpsimd.tensor_mul(sqr[:], mr[:], mr[:])
                pt, r0, mt = ph_slot(ph, 1)
                nc.tensor.matmul(pt[r0:r0 + 1], _mm_in(ON64[:]), _mm_in(sqr[:]),
                                 start=True, stop=True)
                return dict(ctx=ctx, n=n, mr=mr, r0=r0, mt=mt)

            def up_root_post(st):
                r0 = st["r0"]
                pb = pnrm.tile([64, CB], F32, tag="N", name="pbr")
                nc.tensor.matmul(pb[:], _mm_in(ONESBX[r0:r0 + 1, 0:64]),
                                 _mm_in(st["mt"][r0:r0 + 1]),
                                 start=True, stop=True)
                nc.vector.tensor_mul(st["ctx"]["MU"][st["n"]][:], st["mr"][:],
                                     pb[:])

            def dn_stage1a(ctx, n, slot):
                """MD-independent xa1 tanh, pre-issued before the previous
                step's norm phase so the scalar engine has it done (and the
                xa1-half l1 matmuls ready) right at the phase boundary."""
                CS = ctx["CS"]
                p = PARENTS[n]
                rcs = ctx["rootcs"] if p < 0 else CS[n]
                xa1 = tp.tile([107, CB], MDT, tag=f"xa1_{slot % 2}",
                              name="xa1", bufs=2)
                nc.scalar.activation(xa1[:], rcs[:], AF.Tanh)
                return dict(ctx=ctx, n=n, slot=slot, xa1=xa1)

            def dn_stage1(pre):
                """xa2 tanhs + l1 matmuls + h1 relu drains."""
                ctx, n, slot = pre["ctx"], pre["n"], pre["slot"]
                xa1 = pre["xa1"]
                MU, MD = ctx["MU"], ctx["MD"]
                leaf = n in LEAVES
                p = PARENTS[n]
                xa2 = tp.tile([128, CB], MDT, tag=f"xa2_{slot % 2}",
                              name="xa2", bufs=2)
                nc.scalar.activation(xa2[0:64], MU[n][:], AF.Tanh)
                if p >= 0:
                    mi = MD[p][64 * SLOT[n]: 64 * SLOT[n] + 64]
                    nc.scalar.activation(xa2[64:128], mi, AF.Tanh)
                    k2 = 128
                else:
                    k2 = 64  # root: msg_in = 0 contributes nothing
                hn = None
                if not leaf:
                    # fp8 copy of the l1 input for the msg head (DoubleRow)
                    xtag = "xa_root" if p < 0 else f"xaq_{slot % 2}"
                    xaq = tp.tile([128, 2, CB], FP8, tag=xtag, name=xtag, bufs=2)
                    zpad = [xaq[:, 1, :]]
                    if p < 0:
                        zpad.append(xaq[:, 0, :])
                    zonce(xtag, zpad)
                    nc.vector.tensor_copy(xaq[0:k2, 0, :], xa2[0:k2])
                    nc.vector.tensor_copy(xaq[0:107, 1, :], xa1[:])

                # l1 act head: bf16, 448 packed cols (j0-j2 + 64-wide j3)
                h1 = []
                for j in range(4):
                    mj = 128 if j < 3 else 64
                    pq = pbig.tile([mj, CB], F32, tag="P", name="ph1")
                    cols = slice(128 * j, 128 * j + mj)
                    nc.tensor.matmul(pq[:], _mm_in(WL1AB[:, cols]), _mm_in(xa1[:]),
                                     start=True, stop=False)
                    nc.tensor.matmul(pq[:], _mm_in(WL1BB[0:k2, cols]),
                                     _mm_in(xa2[0:k2]), start=False, stop=True)
                    rows = 128 if j < 3 else 32
                    h = tp.tile([rows, CB], MDT, tag=f"h1_{j}", name=f"h1_{j}",
                                bufs=2)
                    nc.vector.tensor_single_scalar(h[:], pq[0:rows], 0.0, OP.max)
                    h1.append(h)
                    if j == 3 and not leaf:
                        hn = tp.tile([128, 2, CB], FP8, tag=f"h1n_{slot % 2}",
                                     name="h1n", bufs=2)
                        zonce(f"h1n_{slot % 2}", [hn[:, 1, :]])
                        nc.vector.tensor_single_scalar(hn[0:32, 1, :], pq[32:64],
                                                       0.0, OP.max)

                # l1 msg head: fp8 DoubleRow over j4-j6
                hm = None
                if not leaf:
                    hm = tp.tile([128, 2, CB], FP8, tag=f"h1m_{slot % 2}",
                                 name="h1m", bufs=2)
                    for j in range(3):
                        pq = pbig.tile([128, CB], F32, tag="P", name="ph1m")
                        cols = slice(128 * j, 128 * j + 128)
                        nc.tensor.matmul(pq[:], _mm_in(WL1DRM[:, :, cols]),
                                         _mm_in(xaq[:, :, :]),
                                         start=True, stop=True, perf_mode=DRPM)
                        dst = hm[:, j, :] if j < 2 else hn[:, 0, :]
                        if j < 2:
                            nc.scalar.activation(dst, pq[:], AF.Relu)
                        else:
                            nc.vector.tensor_single_scalar(dst, pq[:], 0.0,
                                                           OP.max)
                return dict(ctx=ctx, n=n, slot=slot, leaf=leaf, h1=h1,
                            hm=hm, hn=hn)

            def dn_stage2(st):
                """l2 matmuls + h2 relu drains."""
                h1, leaf = st["h1"], st["leaf"]
                h2 = {0: [], 1: []}
                aK = [h1[0][:], h1[1][:], h1[2][:], h1[3][:]]
                for i in range(3):
                    mi_ = H2_KS[i]
                    pq = pbig.tile([mi_, CB], F32, tag="P", name="ph2")
                    cols = slice(128 * i, 128 * i + mi_)
                    for kk in range(4):
                        nc.tensor.matmul(pq[:], _mm_in(WL2A[kk][:, cols]),
                                         _mm_in(aK[kk]),
                                         start=(kk == 0), stop=(kk == 3))
                    h = tp.tile([mi_, CB], MDT, tag=f"h2_0_{i}",
                                name=f"h2_0_{i}", bufs=2)
                    nc.scalar.activation(h[:], pq[:], AF.Relu)
                    h2[0].append(h)
                # l2 msg head: fp8 DoubleRow
                if not leaf:
                    hm, hn = st["hm"], st["hn"]
                    for i in range(3):
                        mi_ = H2_KS[i]
                        pq = pbig.tile([128, CB], F32, tag="P", name="ph2m")
                        cols = slice(128 * i, 128 * i + 128)
                        nc.tensor.matmul(pq[:], _mm_in(WL2DRM[0][:, :, cols]),
                                         _mm_in(hm[:, :, :]),
                                         start=True, stop=False, perf_mode=DRPM)
                        nc.tensor.matmul(pq[:], _mm_in(WL2DRM[1][:, :, cols]),
                                         _mm_in(hn[:, :, :]),
                                         start=False, stop=True, perf_mode=DRPM)
                        h = tp.tile([mi_, CB], MDT, tag=f"h2_1_{i}",
                                    name=f"h2_1_{i}", bufs=2)
                        nc.vector.tensor_single_scalar(h[:], pq[0:mi_], 0.0,
                                                       OP.max)
                        h2[1].append(h)
                st["h2"] = h2

            def dn_stage3(st):
                """msg l3 matmuls + raw-mdr drain (the action l3 chains are
                batched across the level into concurrent col-quadrants)."""
                h2, slot = st["h2"], st["slot"]
                if st["leaf"]:
                    return None
                pm = pbig.tile([128, CB], F32, tag="P", name="pm")
                for i in range(3):
                    nc.tensor.matmul(pm[:], _mm_in(WL3M[i][:]), _mm_in(h2[1][i][:]),
                                     start=(i == 0), stop=(i == 2))
                mdr = tp.tile([128, CB], MDT, tag=f"mdr{slot}", name=f"mdr{slot}")
                nc.vector.tensor_copy(mdr[:], pm[:])
                return mdr

            def down_level_main(ctx, nodes, ph, pre0=None):
                """One tree level of the top-down pass, stage-interleaved
                across the level's nodes so each node's l1-drain latency is
                covered by its siblings' independent matmuls; the rsqrts
                land in the step's shared ACT phase."""
                paA = psm.tile([98, CB], F32, tag="S", name="paA")
                paB = psm.tile([98, CB], F32, tag="S", name="paB") \
                    if len(nodes) > 3 else None
                # skewed wavefront (stage j of node i at key i + 2(j-1)) —
                # deep enough for sibling overlap, shallow enough that each
                # bufs=2 tag's previous readers are emitted before re-alloc
                sts = [None] * len(nodes)
                mdrs = []
                jobs = sorted(
                    [(i + 2 * (j - 1), -j, i, j)
                     for i in range(len(nodes)) for j in (1, 2, 3)])
                for _, _, i, j in jobs:
                    if j == 1:
                        pre = pre0 if (i == 0 and pre0 is not None) \
                            else dn_stage1a(ctx, nodes[i], i)
                        sts[i] = dn_stage1(pre)
                    elif j == 2:
                        dn_stage2(sts[i])
                    else:
                        pa = paA if i < 3 else paB
                        r0 = 32 * (i if i < 3 else i - 3)
                        for li3 in range(3):
                            nc.tensor.matmul(pa[r0:r0 + 1],
                                             _mm_in(WL3A[li3][:]),
                                             _mm_in(sts[i]["h2"][0][li3][:]),
                                             start=(li3 == 0), stop=(li3 == 2))
                        mdr = dn_stage3(sts[i])
                        if mdr is not None:
                            mdrs.append((sts[i]["n"], mdr))
                # batched action drain: one tanh per pa tile, then row DMAs
                nA = min(len(nodes), 3)
                arowt = tp.tile([98, CB], F32, tag="arow", name="arow", bufs=2)
                nc.scalar.activation(arowt[0:32 * (nA - 1) + 1],
                                     paA[0:32 * (nA - 1) + 1], AF.Tanh)
                if paB is not None:
                    nc.scalar.activation(arowt[96:97], paB[0:1], AF.Tanh)
                for slot, n in enumerate(nodes):
                    asrc = arowt[32 * slot:32 * slot + 1] if slot < 3 \
                        else arowt[96:97]
                    nc.sync.dma_start(outt[n:n + 1, ctx["ccols"]], asrc)
                st = dict(ctx=ctx, mdrs=mdrs, slots=[])
                for k, (n, mdr) in enumerate(mdrs):
                    sqd = tp.tile([128, CB], MDT, tag="sqd", name="sqd", bufs=2)
                    nc.gpsimd.tensor_mul(sqd[:], mdr[:], mdr[:])
                    pt, r0, mt = ph_slot(ph, 1)
                    st["slots"].append((r0, mt))
                    nc.tensor.matmul(pt[r0:r0 + 1], _mm_in(ON128[:]),
                                     _mm_in(sqd[:]), start=True, stop=True)
                return st

            def down_level_post(st):
                MD = st["ctx"]["MD"]
                for k, (n, mdr) in enumerate(st["mdrs"]):
                    rq, nt = st["slots"][k]
                    pb = pnrm.tile([128, CB], F32, tag="N", name="pbd")
                    nc.tensor.matmul(pb[:], _mm_in(ONESBX[rq:rq + 1]),
                                     _mm_in(nt[rq:rq + 1]),
                                     start=True, stop=True)
                    nc.vector.tensor_mul(MD[n][:], mdr[:], pb[:])

            def alloc_state(ctx):
                # mu/md tiles are (re)allocated right before the chunk's
                # up-pass; bufs=2 keeps the previous chunk's versions
                # readable through its (interleaved) down-pass
                ctx["MU"] = [pp.tile([64, CB], MDT, tag=f"mu{n}", name=f"mu{n}",
                                     bufs=2) for n in range(NL)]
                ctx["MD"] = {n: (pp.tile([128, CB], MDT, tag=f"md{n}",
                                         name=f"md{n}", bufs=2)
                                 if n not in LEAVES else None)
                             for n in range(NL)}

            # Software pipeline: the down-pass of chunk c is interleaved at
            # tree-level granularity with the up-pass of chunk c+1 (and the
            # front of chunk c+2 at the tail).  Each interleave step ends in
            # ONE shared Abs_reciprocal_sqrt ACT phase covering both tracks'
            # norms, so the sqrt table is loaded ~5x per chunk instead of
            # ~10x, and each track's serial norm latency is filled with the
            # other track's independent matmuls (keeps the PE HAM-warm).
            ph = ph_new()
            ctx = emit_front_mm(0, ph)
            ph_rsqrt(ph)
            emit_front_post(ctx)
            emit_front_b(ctx)
            alloc_state(ctx)
            for grp in UP_GROUPS:  # prologue: up(0) un-interleaved
                ph = ph_new()
                st = up_group_mm(ctx, grp, ph)
                ph_rsqrt(ph)
                up_group_post(st)
            for n in (1, 0):
                ph = ph_new()
                st = up_root_main(ctx, n, ph)
                ph_rsqrt(ph)
                up_root_post(st)
            nxt = None
            if nch > 1:
                ph = ph_new()
                nxt = emit_front_mm(1, ph)
                ph_rsqrt(ph)
                emit_front_post(nxt)
                emit_front_b(nxt)

            USTEPS = [("grp", UP_GROUPS[0]), ("grp", UP_GROUPS[1]),
                      ("grp", UP_GROUPS[2]), ("root", 1), ("root", 0)]
            pre_next = dn_stage1a(ctx, 0, 0)
            for c in range(nch):
                if nxt is not None:
                    alloc_state(nxt)
                nxt2 = None
                for li, lev in enumerate(DOWN_LEVELS):
                    ph = ph_new()
                    dst = down_level_main(ctx, lev, ph, pre0=pre_next)
                    pre_next = None
                    ust = None
                    if nxt is not None:
                        kind, arg = USTEPS[li]
                        ust = (up_group_mm(nxt, arg, ph) if kind == "grp"
                               else up_root_main(nxt, arg, ph))
                    fr = None
                    if li == 4 and c + 2 < nch:
                        fr = nxt2 = emit_front_mm(c + 2, ph)
                    # pre-issue the next level's first node's xa1 work so
                    # the PE queue has MD-independent matmuls at the norm
                    # phase boundary (else it head-of-line blocks on the
                    # rsqrt-dependent broadcasts)
                    if li < 4:
                        pre_next = dn_stage1a(ctx, DOWN_LEVELS[li + 1][0], 0)
                    elif nxt is not None:
                        pre_next = dn_stage1a(nxt, 0, 0)
                    # shared rsqrt phase for both tracks (+ next front)
                    ph_rsqrt(ph)
                    with tc.high_priority(offset=-40):
                        down_level_post(dst)
                        if ust is not None:
                            (up_group_post if USTEPS[li][0] == "grp"
                             else up_root_post)(ust)
                    if fr is not None:
                        emit_front_post(fr)
                        emit_front_b(fr)
                ctx, nxt = nxt, nxt2

    nc.compile()
    return nc


def pack_inputs(inputs, shard):
    """Build the in_map for one core given its state shard [n, 132].

    All biases are folded into the weight matrices as extra contraction rows
    multiplying constant-1 activations (SN row 11; cs row 35 = 10 -> tanh = 1;
    h1[3] rows 16/48 = 1; h2 rows 64/44 = 1), so on-chip drains are pure
    copy / relu ops.
    """
    f = np.float32
    sel = lambda a: np.ascontiguousarray(a, dtype=f)

    def pad_rel(w12):  # [12, X] rel-ordered rows -> [107, X] padded cs layout
        r = np.zeros((107, w12.shape[1]), f)
        r[0:3] = w12[0:3]      # |d|
        r[32:35] = w12[3:6]    # d
        r[64:67] = w12[6:9]    # cur(=to) pos
        r[96:99] = w12[9:12]   # from pos
        return r

    watt = inputs["up_att_w"]          # [20, 64]
    wattp = pad_rel(watt[0:12])
    wattp[99:107] = watt[12:20]        # child state rest

    w1 = np.zeros((12, 64), f)
    w1[0:11] = inputs["up_fc1_w"]; w1[11] = inputs["up_fc1_b"]
    # pair-packed fc1: nodes (2k, 2k+1) block-diag with a shared ones row
    w1p = np.zeros((23, 128), f)
    w1p[0:11, 0:64] = inputs["up_fc1_w"]
    w1p[11:22, 64:128] = inputs["up_fc1_w"]
    w1p[22, 0:64] = inputs["up_fc1_b"]
    w1p[22, 64:128] = inputs["up_fc1_b"]
    w2full = np.zeros((128, 128), f); w2full[:, 0:64] = inputs["up_fc2_w"]
    w2dr = np.stack([w2full[0:64], w2full[64:128]], axis=1)  # [64, 2, 128]
    b2e = np.zeros((65, 1), f); b2e[0:64, 0] = inputs["up_fc2_b"]; b2e[64, 0] = 20.0
    w3 = np.zeros((65, 64), f)
    w3[0:64] = inputs["up_fc3_w"]; w3[64] = inputs["up_fc3_b"]
    # pair-packed fc2/fc3: two nodes block-diag on a [128]-row tile
    # (node-even at rows/cols 0:64, node-odd at 64:128); fc3 bias applied
    # at the PSUM drain instead of a contraction one-row
    w2p = np.zeros((128, 2, 128), f)
    for h in (0, 64):
        w2p[h:h + 64, 0, h:h + 64] = inputs["up_fc2_w"][0:64]
        w2p[h:h + 64, 1, h:h + 64] = inputs["up_fc2_w"][64:128]
    b2p = np.tile(inputs["up_fc2_b"], 2).reshape(128, 1).astype(f)
    w3p = np.zeros((128, 128), f)
    w3p[0:64, 0:64] = inputs["up_fc3_w"]
    w3p[64:128, 64:128] = inputs["up_fc3_w"]
    b3p = np.tile(inputs["up_fc3_b"], 2).reshape(128, 1).astype(f)

    aw1, mw1 = inputs["act_l1_w"], inputs["msg_l1_w"]      # [140,400] each
    ab1, mb1 = inputs["act_l1_b"], inputs["msg_l1_b"]      # [400]
    # 896 packed l1 out cols (7 x 128): act[0:384] | j3(128) | msg[0:384]
    # j3: act384:400 @0:16, one-col @16, msg384:400 @32:48, one-col @48
    def pack_cols(wa, wm):
        blk = np.zeros((wa.shape[0], 128), wa.dtype)
        blk[:, 0:16] = wa[:, 384:400]
        blk[:, 32:48] = wm[:, 384:400]
        return np.concatenate([wa[:, 0:384], blk, wm[:, 0:384]], axis=1)

    wl1 = pack_cols(aw1, mw1)          # [140, 896]
    wl1a = pad_rel(wl1[0:12])          # [107, 896]
    # bias row (row 35; multiplied by tanh(10) == 1.0 in xa group1)
    brow = np.zeros(896, f)
    brow[0:384] = ab1[0:384]
    brow[384:400] = ab1[384:400]
    brow[400] = 1.0                    # h1[3] row16 := 1 (act l2 bias row)
    brow[416:432] = mb1[384:400]
    brow[432] = 1.0                    # h1[3] row48 := 1 (msg l2 bias row)
    brow[512:896] = mb1[0:384]
    wl1a[35] = brow
    # bf16 act-head l1 = first 448 cols; fp8 DoubleRow msg l1 = cols 512:896
    wl1a_b = wl1a[:, 0:448]
    wl1b_b = wl1[12:140, 0:448]
    wl1drm = np.zeros((128, 2, 384), f)
    wl1drm[:, 0, :] = wl1[12:140, 512:896]
    wl1drm[0:107, 1, :] = wl1a[:, 512:896]

    a2, m2 = inputs["act_l2_w"], inputs["msg_l2_w"]
    ab2, mb2 = inputs["act_l2_b"], inputs["msg_l2_b"]
    a3, m3 = inputs["act_l3_w"], inputs["msg_l3_w"]

    def l2dr(w, b):
        """Two DoubleRow k-chunks [128, 2, 384] for one 400->300 layer;
        chunk1 group1 = tail rows 384:400 + bias row (one-col at 300)."""
        c0 = np.zeros((128, 2, 384), f)
        c0[:, 0, 0:300] = w[0:128]
        c0[:, 1, 0:300] = w[128:256]
        c1 = np.zeros((128, 2, 384), f)
        c1[:, 0, 0:300] = w[256:384]
        c1[0:16, 1, 0:300] = w[384:400]
        c1[16, 1, 0:300] = b
        c1[16, 1, 300] = 1.0           # h2 one-row (l3 bias row)
        return c0, c1

    def l2tiles_act(w, b):
        outs = {}
        for i, (r0, r1) in enumerate(((0, 128), (128, 256), (256, 384))):
            t = np.zeros((128, 301), f)
            t[:, 0:300] = w[r0:r1]
            outs[i] = t
        t = np.zeros((32, 301), f)
        t[0:16, 0:300] = w[384:400]
        t[16, 0:300] = b
        t[16, 300] = 1.0               # h2 one-row (l3 bias row)
        outs[3] = t
        return outs

    l2a = l2tiles_act(a2, ab2)
    l2m = l2dr(m2, mb2)

    selb2 = (np.arange(128)[None, :] // 64 == np.arange(2)[:, None]).astype(f)
    selb2x = np.zeros((98, 128), f)
    for k in range(4):
        selb2x[32 * k:32 * k + 2] = selb2
    im = {
        "statet": np.ascontiguousarray(shard.T, dtype=f),
        "w1": w1, "w1p": w1p,
        "wattp": wattp, "wattn": -wattp,
        "w2dr": w2dr, "b2e": b2e, "w3": w3,
        "w2p": w2p, "b2p": b2p, "w3p": w3p, "b3p": b3p,
        "ones64": np.ones((64, 1), f), "onesb64": np.ones((1, 64), f),
        "ones128": np.ones((128, 1), f),
        "sel2": (np.arange(128)[:, None] // 64 == np.arange(2)[None, :]).astype(f),
        "selb2x": selb2x,
        "onesbx": np.ones((98, 128), f),
        "wl1a_b": wl1a_b, "wl1b_b": wl1b_b,
        "wl1drm": wl1drm,
        "wl2a0": l2a[0], "wl2a1": l2a[1], "wl2a2": l2a[2], "wl2a3": l2a[3],
        "wl2drm0": l2m[0], "wl2drm1": l2m[1],
    }
    for i, (r0, r1) in enumerate(((0, 128), (128, 256), (256, 300))):
        if i < 2:
            im[f"wl3a{i}"] = sel(a3[r0:r1])
            im[f"wl3m{i}"] = sel(m3[r0:r1])
        else:
            t = np.zeros((45, 1), f); t[0:44] = a3[256:300]; t[44] = inputs["act_l3_b"]
            im["wl3a2"] = t
            t = np.zeros((45, 128), f); t[0:44] = m3[256:300]; t[44] = inputs["msg_l3_b"]
            im["wl3m2"] = t
    import ml_dtypes
    FP8_KEYS = ("w2dr", "wl1drm", "wl2drm0", "wl2drm1", "w2p")
    for k in im:
        if k in ("b2e", "b2p", "b3p"):
            continue
        if k in FP8_KEYS:
            im[k] = im[k].astype(ml_dtypes.float8_e4m3)
        else:
            im[k] = im[k].astype(ml_dtypes.bfloat16 if MM_DT != "f32" else f)
    return im


_CACHED_NC = None


def _run(inputs, trace=False, **kw):
    global _CACHED_NC
    if _CACHED_NC is None:
        _CACHED_NC = build_program()
    nc = _CACHED_NC
    state = np.asarray(inputs["state"], dtype=np.float32)
    n = NCH * CB
    in_maps = [pack_inputs(inputs, state[i * BLOC: i * BLOC + n]) for i in range(NCORES)]
    res = run_bass_kernel_spmd(nc, in_maps, core_ids=list(range(NCORES)),
                               trace=trace, **kw)
    outs = [np.asarray(res.results[i]["outt"]).T for i in range(NCORES)]
    return np.concatenate(outs, axis=0).astype(np.float32), res


def kernel(**inputs):
    return _run(inputs)[0]

